# revision 6
# baseline (speedup 1.0000x reference)
"""Trainium2 Bass kernel for nn_AttentionBlock (b=16, c=32, 128x128 spatial,
heads=8, dim_head=64).

Sharding: sequence-parallel over the flattened spatial dim N=16384 across 8
NeuronCores (2048 positions per core). Projections are per-position so they
shard exactly; the QK^T reduction over N is computed as per-core partials
followed by per-group bf16 AllReduces of sim (64KB each), pipelined against
the v projection and the first group's output gemm; softmax is replicated.

Per-core layouts (SBUF partition dim first):
  x_bf[g][cc][p]  [128=(4 batch x 32 ch), 512=m]  bf16  (16 pieces)
  wqT/wkT/wvT[cc] [128=c-chunk, 512=f]            bf16  (wqT has 1/8 folded in)
  woT[fc]         [128=f-chunk=(2 heads x 64 n), 256=o] bf16
  q/k (transient) [128=m-chunk, 512=f]            bf16
  sim psum[g]     [128=(head parity,i), 4*128=(head pair, j)] f32
  v_sb[g][ft]     [128=f-chunk=(parity,j), 2048=m] bf16
  At[g][fc]       [128=f-chunk=(parity,j), 256=o] bf16  (Wout^T @ attn fused)
  out psum        [128=o-chunk, 512=m]            f32   (+bout bias on copy out)
"""

import numpy as np

N_CORES = 8
B, C, HS, WS = 16, 32, 128, 128
N = HS * WS              # 16384
NS = N // N_CORES        # 2048 per-core spatial shard
H = 8                    # heads
DH = 64                  # dim_head
QD = H * C               # 256 linear in_features
INNER = H * DH           # 512
G = B // H               # 2 groups
SCALE = DH ** -0.5       # 0.125
MT = NS // 128           # 16 m-chunks of 128
NP = NS // 512           # 4 pieces of 512

_CACHE = {}


def _build_nc():
    import concourse.bacc as bacc
    import concourse.mybir as mybir
    import concourse.tile as tile
    from concourse import masks
    from contextlib import ExitStack

    f32 = mybir.dt.float32
    bf16 = mybir.dt.bfloat16
    RG = [list(range(N_CORES))]

    nc = bacc.Bacc("TRN2", target_bir_lowering=False, debug=False,
                   num_devices=N_CORES)

    x_ext = nc.dram_tensor("x", [B, C, NS], f32, kind="ExternalInput")
    wq_ext = nc.dram_tensor("Wq", [INNER, QD], f32, kind="ExternalInput")
    wkv_ext = nc.dram_tensor("Wkv", [2 * INNER, QD], f32, kind="ExternalInput")
    wo_ext = nc.dram_tensor("Wout", [QD, INNER], f32, kind="ExternalInput")
    bout_ext = nc.dram_tensor("bout", [QD], f32, kind="ExternalInput")
    out_ext = nc.dram_tensor("out", [B, C, NS], f32, kind="ExternalOutput")

    # collective bounce buffers (HBM; outs in Shared space for the fast path)
    warm_in = nc.dram_tensor("warm_in", [128, 4], f32)
    warm_out = nc.dram_tensor("warm_out", [128, 4], f32, addr_space="Shared")
    ar_in = [nc.dram_tensor(f"ar_in{g}", [128, 256], f32) for g in range(G)]
    ar_out = [nc.dram_tensor(f"ar_out{g}", [128, 256], f32,
                             addr_space="Shared") for g in range(G)]

    with tile.TileContext(nc) as tc:
        with ExitStack() as ctx:
            persist = ctx.enter_context(tc.tile_pool(name="persist", bufs=1))

            # ---- warmup collective: absorbs inter-core skew during startup
            warm_sb = persist.tile([128, 4], f32, tag="warm_sb")
            nc.gpsimd.memset(warm_sb[:], 0.0)
            nc.sync.dma_start(warm_in[:], warm_sb[:])
            nc.gpsimd.collective_compute(
                "AllReduce", mybir.AluOpType.add, replica_groups=RG,
                ins=[warm_in[:]], outs=[warm_out[:]])

            ident = persist.tile([128, 128], f32, tag="ident")
            masks.make_identity(nc, ident[:])

            # ---- weight staging DMAs ----
            wq_st = persist.tile([128, 4, 256], f32, tag="wq_st")
            nc.sync.dma_start(wq_st[:], wq_ext[:].rearrange("(t p) c -> p t c", p=128))
            wkv_st = persist.tile([128, 8, 256], f32, tag="wkv_st")
            nc.sync.dma_start(wkv_st[:], wkv_ext[:].rearrange("(t p) c -> p t c", p=128))
            wo_st = persist.tile([128, 2, 512], f32, tag="wo_st")
            nc.sync.dma_start(wo_st[:], wo_ext[:].rearrange("(t p) c -> p t c", p=128))
            bout_sb = persist.tile([128, 2], f32, tag="bout_sb")
            nc.sync.dma_start(bout_sb[:], bout_ext[:].rearrange("(t p) -> p t", p=128))

            # ---- x load (16 pieces) + f32->bf16 convert on GpSimd ----
            x_bf = [[[None] * NP for _ in range(2)] for _ in range(G)]
            with tc.tile_pool(name="xstage", bufs=4) as xstage:
                for g in range(G):
                    for cc in range(2):
                        b0 = g * 8 + cc * 4
                        for p in range(NP):
                            xs = xstage.tile([128, 512], f32, tag="xs")
                            nc.sync.dma_start(
                                xs[:],
                                x_ext[b0:b0 + 4, :, p * 512:(p + 1) * 512]
                                .rearrange("a b m -> (a b) m"))
                            xb = persist.tile([128, 512], bf16,
                                              tag=f"xbf{g}{cc}{p}",
                                              name=f"xbf{g}{cc}{p}")
                            nc.gpsimd.tensor_copy(xb[:], xs[:])
                            x_bf[g][cc][p] = xb

            # ---- transpose weights on PE (q/k first so the qk loop starts
            # early; 1/8 attention scale folded into wqT) ----
            wqT = [persist.tile([128, 512], bf16, tag=f"wqT{cc}", name=f"wqT{cc}")
                   for cc in range(2)]
            wkT = [persist.tile([128, 512], bf16, tag=f"wkT{cc}", name=f"wkT{cc}")
                   for cc in range(2)]
            wvT = [persist.tile([128, 512], bf16, tag=f"wvT{cc}", name=f"wvT{cc}")
                   for cc in range(2)]
            woT = [persist.tile([128, 256], bf16, tag=f"woT{fc}", name=f"woT{fc}")
                   for fc in range(4)]
            with tc.tile_pool(name="tps", bufs=4, space="PSUM") as tps:
                for cc in range(2):
                    cs = slice(cc * 128, (cc + 1) * 128)
                    for t in range(4):
                        pt = tps.tile([128, 128], f32, tag="tp")
                        nc.tensor.transpose(pt[:], wq_st[:, t, cs], ident[:])
                        nc.scalar.mul(wqT[cc][:, t * 128:(t + 1) * 128], pt[:],
                                      SCALE)
                    for t in range(4):
                        pt = tps.tile([128, 128], f32, tag="tp")
                        nc.tensor.transpose(pt[:], wkv_st[:, t, cs], ident[:])
                        nc.scalar.copy(wkT[cc][:, t * 128:(t + 1) * 128], pt[:])
                for cc in range(2):
                    cs = slice(cc * 128, (cc + 1) * 128)
                    for t in range(4):
                        pt = tps.tile([128, 128], f32, tag="tp")
                        nc.tensor.transpose(pt[:], wkv_st[:, t + 4, cs], ident[:])
                        nc.scalar.copy(wvT[cc][:, t * 128:(t + 1) * 128], pt[:])
                for oc in range(2):
                    for fj in range(4):
                        pt = tps.tile([128, 128], f32, tag="tp")
                        nc.tensor.transpose(
                            pt[:], wo_st[:, oc, fj * 128:(fj + 1) * 128], ident[:])
                        nc.scalar.copy(woT[fj][:, oc * 128:(oc + 1) * 128], pt[:])

            # ---- q/k projections + sim partials; per-group AllReduce ----
            simsb = [persist.tile([128, 4, 64], f32, tag=f"simsb{g}",
                                  name=f"simsb{g}") for g in range(G)]
            with tc.tile_pool(name="simps", bufs=1, space="PSUM") as simpool, \
                 tc.tile_pool(name="qkps", bufs=2, space="PSUM") as qkps, \
                 tc.tile_pool(name="qksb", bufs=3) as qksb:
                sim_ps = [simpool.tile([128, 512], f32, tag=f"sim{g}",
                                       name=f"sim{g}") for g in range(G)]
                for g in range(G):
                    for mt in range(MT):
                        p, lo = mt // 4, (mt % 4) * 128
                        ms = slice(lo, lo + 128)
                        qp = qkps.tile([128, 512], f32, tag="qp")
                        kp = qkps.tile([128, 512], f32, tag="kp")
                        for cc in range(2):
                            nc.tensor.matmul(qp[:], x_bf[g][cc][p][:, ms],
                                             wqT[cc][:],
                                             start=(cc == 0), stop=(cc == 1))
                            nc.tensor.matmul(kp[:], x_bf[g][cc][p][:, ms],
                                             wkT[cc][:],
                                             start=(cc == 0), stop=(cc == 1))
                        q_t = qksb.tile([128, 512], bf16, tag="q_t")
                        k_t = qksb.tile([128, 512], bf16, tag="k_t")
                        nc.vector.tensor_copy(q_t[:], qp[:])
                        if mt % 2 == 0:
                            nc.vector.tensor_copy(k_t[:], kp[:])
                        else:
                            nc.scalar.copy(k_t[:], kp[:])
                        # single accumulation group per bank: only the first
                        # matmul starts (zeroes the 2KB zero region), only
                        # the last stops
                        for hp in range(4):
                            hs = slice(hp * 128, (hp + 1) * 128)
                            nc.tensor.matmul(sim_ps[g][:, hs], q_t[:, hs],
                                             k_t[:, hs],
                                             start=(mt == 0 and hp == 0),
                                             stop=(mt == MT - 1 and hp == 3))
                    # extract diagonal 64x64 blocks, cast bf16, AllReduce
                    for hp in range(4):
                        nc.vector.tensor_copy(
                            simsb[g][0:64, hp, :],
                            sim_ps[g][0:64, hp * 128:hp * 128 + 64])
                        nc.vector.tensor_copy(
                            simsb[g][64:128, hp, :],
                            sim_ps[g][64:128, hp * 128 + 64:hp * 128 + 128])
                    nc.sync.dma_start(ar_in[g][:],
                                      simsb[g][:].rearrange("p s j -> p (s j)"))
                    nc.gpsimd.collective_compute(
                        "AllReduce", mybir.AluOpType.add, replica_groups=RG,
                        ins=[ar_in[g][:]], outs=[ar_out[g][:]])

            attn_in = [persist.tile([128, 4, 64], f32, tag=f"attn_in{g}",
                                    name=f"attn_in{g}") for g in range(G)]
            for g in range(G):
                nc.sync.dma_start(attn_in[g][:].rearrange("p s j -> p (s j)"),
                                  ar_out[g][:])

            # ---- v projection (independent of the AllReduces) ----
            v_sb = [[persist.tile([128, NS], bf16, tag=f"v{g}{ft}",
                                  name=f"v{g}{ft}") for ft in range(4)]
                    for g in range(G)]
            with tc.tile_pool(name="vps", bufs=2, space="PSUM") as vps:
                for g in range(G):
                    for ft in range(4):
                        fs = slice(ft * 128, (ft + 1) * 128)
                        for mt4 in range(NP):
                            ms = slice(mt4 * 512, (mt4 + 1) * 512)
                            vp = vps.tile([128, 512], f32, tag="vp")
                            for cc in range(2):
                                nc.tensor.matmul(vp[:], wvT[cc][:, fs],
                                                 x_bf[g][cc][mt4][:],
                                                 start=(cc == 0), stop=(cc == 1))
                            if mt4 % 2 == 0:
                                nc.vector.tensor_copy(v_sb[g][ft][:, ms], vp[:])
                            else:
                                nc.scalar.copy(v_sb[g][ft][:, ms], vp[:])

            # ---- per-group: softmax -> At -> final gemm -> out DMA ----
            with tc.tile_pool(name="smx", bufs=1) as smx, \
                 tc.tile_pool(name="aps", bufs=2, space="PSUM") as aps, \
                 tc.tile_pool(name="ops", bufs=2, space="PSUM") as ops, \
                 tc.tile_pool(name="osb", bufs=3) as osb:
                At = [[persist.tile([128, 256], bf16, tag=f"At{g}{fc}",
                                    name=f"At{g}{fc}") for fc in range(4)]
                      for g in range(G)]
                for g in range(G):
                    negmax = smx.tile([128, 4], f32, tag=f"negmax{g}",
                                      name=f"negmax{g}")
                    nc.vector.reduce_max(negmax[:], attn_in[g][:],
                                         axis=mybir.AxisListType.X, negate=True)
                    shifted = smx.tile([128, 4, 64], f32, tag=f"shifted{g}",
                                       name=f"shifted{g}")
                    nc.vector.tensor_add(shifted[:], attn_in[g][:],
                                         negmax[:].broadcast_to([128, 4, 64]))
                    expt = smx.tile([128, 4, 64], f32, tag=f"expt{g}",
                                    name=f"expt{g}")
                    nc.scalar.activation(expt[:], shifted[:],
                                         mybir.ActivationFunctionType.Exp)
                    sums = smx.tile([128, 4], f32, tag=f"sums{g}",
                                    name=f"sums{g}")
                    nc.vector.reduce_sum(sums[:], expt[:],
                                         axis=mybir.AxisListType.X)
                    rsum = smx.tile([128, 4], f32, tag=f"rsum{g}",
                                    name=f"rsum{g}")
                    nc.vector.reciprocal(rsum[:], sums[:])
                    attn_bf = smx.tile([128, 4, 64], bf16, tag=f"attn_bf{g}",
                                       name=f"attn_bf{g}")
                    nc.vector.tensor_mul(attn_bf[:], expt[:],
                                         rsum[:].broadcast_to([128, 4, 64]))

                    # At[g][fc][(parity,j), o] = sum_n attn[n,j] WoutT[f,o]
                    for fc in range(4):
                        ap_t = aps.tile([128, 256], f32, tag="ap_t")
                        for parity in range(2):
                            ps = slice(parity * 64, (parity + 1) * 64)
                            nc.tensor.matmul(ap_t[ps, :], attn_bf[ps, fc, :],
                                             woT[fc][ps, :],
                                             start=True, stop=True)
                        nc.vector.tensor_copy(At[g][fc][:], ap_t[:])

                    # out[o, m] = sum_f At[f, o] * v[f, m]  (+bout)
                    for ot in range(2):
                        os_ = slice(ot * 128, (ot + 1) * 128)
                        for mt4 in range(NP):
                            ms = slice(mt4 * 512, (mt4 + 1) * 512)
                            op_t = ops.tile([128, 512], f32, tag="op_t")
                            for fc in range(4):
                                nc.tensor.matmul(op_t[:], At[g][fc][:, os_],
                                                 v_sb[g][fc][:, ms],
                                                 start=(fc == 0), stop=(fc == 3))
                            o_t = osb.tile([128, 512], f32, tag="o_t")
                            if mt4 % 2 == 0:
                                nc.vector.tensor_scalar_add(
                                    o_t[:], op_t[:], bout_sb[:, ot:ot + 1])
                            else:
                                nc.scalar.activation(
                                    o_t[:], op_t[:],
                                    mybir.ActivationFunctionType.Identity,
                                    bias=bout_sb[:, ot:ot + 1])
                            b0 = g * 8 + ot * 4
                            nc.sync.dma_start(
                                out_ext[b0:b0 + 4, :, ms].rearrange(
                                    "a b m -> (a b) m"),
                                o_t[:])

    nc.compile()
    return nc


def _get_nc():
    if "nc" not in _CACHE:
        _CACHE["nc"] = _build_nc()
    return _CACHE["nc"]


def make_in_maps(x, Wq, Wkv, Wout, bout):
    xf = np.ascontiguousarray(x, dtype=np.float32).reshape(B, C, N)
    Wq = np.ascontiguousarray(Wq, dtype=np.float32)
    Wkv = np.ascontiguousarray(Wkv, dtype=np.float32)
    Wout = np.ascontiguousarray(Wout, dtype=np.float32)
    bout = np.ascontiguousarray(bout, dtype=np.float32)
    return [
        {
            "x": np.ascontiguousarray(xf[:, :, i * NS:(i + 1) * NS]),
            "Wq": Wq, "Wkv": Wkv, "Wout": Wout, "bout": bout,
        }
        for i in range(N_CORES)
    ]


def gather_out(results):
    out = np.concatenate([results[i]["out"] for i in range(N_CORES)], axis=2)
    return out.reshape(B, C, HS, WS).astype(np.float32)


def run_sharded(in_maps, **kw):
    from concourse.bass_utils import run_bass_kernel_spmd
    nc = _get_nc()
    return run_bass_kernel_spmd(nc, in_maps, list(range(N_CORES)), **kw)


def kernel(x, Wq, Wkv, Wout, bout):
    in_maps = make_in_maps(x, Wq, Wkv, Wout, bout)
    res = run_sharded(in_maps)
    return gather_out(res.results)


if __name__ == "__main__":
    nc = _get_nc()
    print("built + compiled OK")


# revision 10
# speedup vs baseline: 1.2318x; 1.2318x over previous
"""Trainium2 Bass kernel for nn_AttentionBlock (b=16, c=32, 128x128 spatial,
heads=8, dim_head=64).

Sharding: sequence-parallel over the flattened spatial dim N=16384 across 8
NeuronCores (2048 positions per core). Projections are per-position so they
shard exactly; the QK^T reduction over N is AllReduced per batch-group (g),
pipelined against the other group's compute; softmax is replicated.

Algebraic structure (per group g, X = x reshaped [256=(4b x 32c), m]):
  S    = X X^T                  (Gram matrix, [256, 256]; m-reduction on PE)
  sim_h = Wq_h (S/8) Wk_h^T     (tiny; q/k never materialized)
  AllReduce(sim) over 8 cores; attn = softmax(sim)
  At[f=(h,j), o] = sum_n attn[h][n, j] Wout[o, h*64+n]
  Mt[c, o] = sum_f Wkv[512+f, c] At[f, o]    (v-projection folded in;
                                              Wv used in natural layout)
  out[o, m] = sum_c Mt[c, o] X[c, m] + bout[o]

Per-core SBUF layouts (partition dim first):
  x_bf[g][cc][p]  [128=(4b x 32c), 512=m] bf16   (16 pieces)
  xT[g][mc]       [128=m, 256=c] bf16            (PE-transposed X)
  wqT/wkT[cc]     [128=c-chunk, 512=f] bf16      (wqT has 1/8 folded in)
  woT[fc]         [128=f-chunk, 256=o] bf16
  S_bf[g][cc]     [128=c1, 256=c2] bf16 (symmetric)
  T1[g][cc]       [128=c1, 512=(h,j)] bf16       (S Wk^T)
  sim psum[g]     [128=(parity,i), 4*128=(pair, j)] f32 (diag 64x64 blocks)
  At[g][fc]       [128=(parity,j), 256=o] bf16
  Mt[g][cc]       [128=c, 256=o] bf16
"""

import numpy as np

N_CORES = 8
B, C, HS, WS = 16, 32, 128, 128
N = HS * WS              # 16384
NS = N // N_CORES        # 2048 per-core spatial shard
H = 8
DH = 64
QD = H * C               # 256
INNER = H * DH           # 512
G = B // H               # 2 groups
SCALE = DH ** -0.5       # 0.125
MT = NS // 128           # 16 m-chunks of 128
NP = NS // 512           # 4 pieces of 512

_CACHE = {}


def _build_nc():
    import concourse.bacc as bacc
    import concourse.mybir as mybir
    import concourse.tile as tile
    from concourse import masks
    from contextlib import ExitStack

    f32 = mybir.dt.float32
    bf16 = mybir.dt.bfloat16
    RG = [list(range(N_CORES))]

    nc = bacc.Bacc("TRN2", target_bir_lowering=False, debug=False,
                   num_devices=N_CORES)

    x_ext = nc.dram_tensor("x", [B, C, NS], f32, kind="ExternalInput")
    wq_ext = nc.dram_tensor("Wq", [INNER, QD], f32, kind="ExternalInput")
    wkv_ext = nc.dram_tensor("Wkv", [2 * INNER, QD], f32, kind="ExternalInput")
    wo_ext = nc.dram_tensor("Wout", [QD, INNER], f32, kind="ExternalInput")
    bout_ext = nc.dram_tensor("bout", [QD], f32, kind="ExternalInput")
    out_ext = nc.dram_tensor("out", [B, C, NS], f32, kind="ExternalOutput")

    warm_in = nc.dram_tensor("warm_in", [128, 4], f32)
    warm_out = nc.dram_tensor("warm_out", [128, 4], f32, addr_space="Shared")
    ar_in = [nc.dram_tensor(f"ar_in{g}", [128, 256], f32) for g in range(G)]
    ar_out = [nc.dram_tensor(f"ar_out{g}", [128, 256], f32,
                             addr_space="Shared") for g in range(G)]

    with tile.TileContext(nc) as tc:
        with ExitStack() as ctx:
            persist = ctx.enter_context(tc.tile_pool(name="persist", bufs=1))

            # ---- warmup collective: absorbs inter-core skew during startup
            warm_sb = persist.tile([128, 4], f32, tag="warm_sb")
            nc.gpsimd.memset(warm_sb[:], 0.0)
            nc.sync.dma_start(warm_in[:], warm_sb[:])
            nc.gpsimd.collective_compute(
                "AllReduce", mybir.AluOpType.add, replica_groups=RG,
                ins=[warm_in[:]], outs=[warm_out[:]])

            ident_f = persist.tile([128, 128], f32, tag="ident_f")
            masks.make_identity(nc, ident_f[:])
            ident_b = persist.tile([128, 128], bf16, tag="ident_b")
            masks.make_identity(nc, ident_b[:])

            # ---- weight staging DMAs ----
            wq_st = persist.tile([128, 4, 256], f32, tag="wq_st")
            nc.sync.dma_start(wq_st[:], wq_ext[:].rearrange("(t p) c -> p t c", p=128))
            wkv_st = persist.tile([128, 8, 256], f32, tag="wkv_st")
            nc.sync.dma_start(wkv_st[:], wkv_ext[:].rearrange("(t p) c -> p t c", p=128))
            wo_st = persist.tile([128, 2, 512], f32, tag="wo_st")
            nc.sync.dma_start(wo_st[:], wo_ext[:].rearrange("(t p) c -> p t c", p=128))
            bout_sb = persist.tile([128, 2], f32, tag="bout_sb")
            nc.sync.dma_start(bout_sb[:], bout_ext[:].rearrange("(t p) -> p t", p=128))

            # ---- x load (16 pieces) + f32->bf16 casts on DVE ----
            x_bf = [[[None] * NP for _ in range(2)] for _ in range(G)]
            with tc.tile_pool(name="xstage", bufs=4) as xstage:
                for g in range(G):
                    for cc in range(2):
                        b0 = g * 8 + cc * 4
                        for p in range(NP):
                            xs = xstage.tile([128, 512], f32, tag="xs")
                            nc.sync.dma_start(
                                xs[:],
                                x_ext[b0:b0 + 4, :, p * 512:(p + 1) * 512]
                                .rearrange("a b m -> (a b) m"))
                            xb = persist.tile([128, 512], bf16,
                                              tag=f"xbf{g}{cc}{p}",
                                              name=f"xbf{g}{cc}{p}")
                            nc.vector.tensor_copy(xb[:], xs[:])
                            x_bf[g][cc][p] = xb

            # v-projection weights in natural [f, c] layout, just cast to bf16
            wv_bf = [persist.tile([128, 256], bf16, tag=f"wv_bf{t}",
                                  name=f"wv_bf{t}") for t in range(4)]
            for t in range(4):
                nc.vector.tensor_copy(wv_bf[t][:], wkv_st[:, 4 + t, :])

            # ---- transposed weights on PE (wq/wk needed for T1/sim; wo for
            # At; 1/8 scale folded into wqT) ----
            wqT = [persist.tile([128, 512], bf16, tag=f"wqT{cc}", name=f"wqT{cc}")
                   for cc in range(2)]
            wkT = [persist.tile([128, 512], bf16, tag=f"wkT{cc}", name=f"wkT{cc}")
                   for cc in range(2)]
            woT = [persist.tile([128, 256], bf16, tag=f"woT{fc}", name=f"woT{fc}")
                   for fc in range(4)]

            simsb = [persist.tile([128, 4, 64], f32, tag=f"simsb{g}",
                                  name=f"simsb{g}") for g in range(G)]
            S_bf = [[persist.tile([128, 256], bf16, tag=f"S{g}{c}",
                                 name=f"S{g}{c}") for c in range(2)]
                    for g in range(G)]
            T1 = [[persist.tile([128, 512], bf16, tag=f"T1{g}{c}",
                                name=f"T1{g}{c}") for c in range(2)]
                  for g in range(G)]
            xT = [[persist.tile([128, 256], bf16, tag=f"xT{g}{mc}",
                                name=f"xT{g}{mc}") for mc in range(MT)]
                  for g in range(G)]

            with tc.tile_pool(name="tps", bufs=2, space="PSUM") as tps, \
                 tc.tile_pool(name="xtps", bufs=2, space="PSUM") as xtps, \
                 tc.tile_pool(name="Sps", bufs=1, space="PSUM") as Sps, \
                 tc.tile_pool(name="simps", bufs=1, space="PSUM") as simpool:

                def w_transposes(rng):
                    # wq/wk chunk transposes (f32 in, bf16 out w/ scale on wq)
                    for cc, t in rng:
                        cs = slice(cc * 128, (cc + 1) * 128)
                        pt = tps.tile([128, 128], f32, tag="tp")
                        nc.tensor.transpose(pt[:], wq_st[:, t, cs], ident_f[:])
                        nc.scalar.mul(wqT[cc][:, t * 128:(t + 1) * 128], pt[:],
                                      SCALE)
                        pt = tps.tile([128, 128], f32, tag="tp")
                        nc.tensor.transpose(pt[:], wkv_st[:, t, cs], ident_f[:])
                        nc.scalar.copy(wkT[cc][:, t * 128:(t + 1) * 128], pt[:])

                def xt_and_S(g):
                    # X^T chunks via PE transpose, S = X X^T accumulation.
                    # S psum reuses one bank across groups (tag-shared).
                    S_ps = Sps.tile([128, 512], f32, tag="Sps",
                                    name=f"Sps{g}")
                    for mc in range(MT):
                        p, lo = mc // 4, (mc % 4) * 128
                        for cc in range(2):
                            pt = xtps.tile([128, 128], bf16, tag="xtp")
                            nc.tensor.transpose(
                                pt[:], x_bf[g][cc][p][:, lo:lo + 128],
                                ident_b[:])
                            if cc == 0:
                                nc.vector.tensor_copy(
                                    xT[g][mc][:, 0:128], pt[:])
                            else:
                                nc.scalar.copy(xT[g][mc][:, 128:256], pt[:])
                        for c1 in range(2):
                            nc.tensor.matmul(
                                S_ps[:, c1 * 256:(c1 + 1) * 256],
                                xT[g][mc][:, c1 * 128:(c1 + 1) * 128],
                                xT[g][mc][:],
                                start=(mc == 0 and c1 == 0),
                                stop=(mc == MT - 1 and c1 == 1))
                    for c1 in range(2):
                        eng_copy = (nc.vector.tensor_copy if c1 == 0
                                    else nc.scalar.copy)
                        eng_copy(S_bf[g][c1][:],
                                 S_ps[:, c1 * 256:(c1 + 1) * 256])

                def sim_and_ar(g, t1ps):
                    # T1 = S Wk^T  -> sim_h = Wq_s,h T1_h -> AllReduce
                    sim_ps = simpool.tile([128, 512], f32, tag="simps",
                                          name=f"sim{g}")
                    for c1 in range(2):
                        t1p = t1ps.tile([128, 512], f32, tag="t1p")
                        for c2 in range(2):
                            nc.tensor.matmul(
                                t1p[:],
                                S_bf[g][c2][:, c1 * 128:(c1 + 1) * 128],
                                wkT[c2][:],
                                start=(c2 == 0), stop=(c2 == 1))
                        if c1 == 0:
                            nc.vector.tensor_copy(T1[g][c1][:], t1p[:])
                        else:
                            nc.scalar.copy(T1[g][c1][:], t1p[:])
                    for hp in range(4):
                        hs = slice(hp * 128, (hp + 1) * 128)
                        for c1 in range(2):
                            nc.tensor.matmul(
                                sim_ps[:, hs], wqT[c1][:, hs],
                                T1[g][c1][:, hs],
                                start=(hp == 0 and c1 == 0),
                                stop=(hp == 3 and c1 == 1))
                    for hp in range(4):
                        nc.vector.tensor_copy(
                            simsb[g][0:64, hp, :],
                            sim_ps[0:64, hp * 128:hp * 128 + 64])
                        nc.vector.tensor_copy(
                            simsb[g][64:128, hp, :],
                            sim_ps[64:128, hp * 128 + 64:hp * 128 + 128])
                    nc.sync.dma_start(ar_in[g][:],
                                      simsb[g][:].rearrange("p s j -> p (s j)"))
                    nc.gpsimd.collective_compute(
                        "AllReduce", mybir.AluOpType.add, replica_groups=RG,
                        ins=[ar_in[g][:]], outs=[ar_out[g][:]])

                with tc.tile_pool(name="t1ps", bufs=2, space="PSUM") as t1ps:
                    xt_and_S(0)
                    w_transposes([(cc, t) for cc in range(2) for t in range(4)])
                    sim_and_ar(0, t1ps)
                    xt_and_S(1)
                    sim_and_ar(1, t1ps)
                # wo transposes (needed post-AR for At)
                for oc in range(2):
                    for fj in range(4):
                        pt = tps.tile([128, 128], f32, tag="tp")
                        nc.tensor.transpose(
                            pt[:], wo_st[:, oc, fj * 128:(fj + 1) * 128],
                            ident_f[:])
                        nc.scalar.copy(woT[fj][:, oc * 128:(oc + 1) * 128],
                                       pt[:])

            attn_in = [persist.tile([128, 4, 64], f32, tag=f"attn_in{g}",
                                    name=f"attn_in{g}") for g in range(G)]
            for g in range(G):
                nc.sync.dma_start(attn_in[g][:].rearrange("p s j -> p (s j)"),
                                  ar_out[g][:])

            # ---- per-group: softmax -> At -> Mt -> final gemm -> out DMA ----
            with tc.tile_pool(name="smx", bufs=1) as smx, \
                 tc.tile_pool(name="aps", bufs=2, space="PSUM") as aps, \
                 tc.tile_pool(name="mps", bufs=2, space="PSUM") as mps, \
                 tc.tile_pool(name="ops", bufs=2, space="PSUM") as ops, \
                 tc.tile_pool(name="osb", bufs=3) as osb:
                At = [[persist.tile([128, 256], bf16, tag=f"At{g}{fc}",
                                    name=f"At{g}{fc}") for fc in range(4)]
                      for g in range(G)]
                Mt = [[persist.tile([128, 256], bf16, tag=f"Mt{g}{cc}",
                                    name=f"Mt{g}{cc}") for cc in range(2)]
                      for g in range(G)]
                for g in range(G):
                    negmax = smx.tile([128, 4], f32, tag=f"negmax{g}",
                                      name=f"negmax{g}")
                    nc.vector.reduce_max(negmax[:], attn_in[g][:],
                                         axis=mybir.AxisListType.X, negate=True)
                    shifted = smx.tile([128, 4, 64], f32, tag=f"shifted{g}",
                                       name=f"shifted{g}")
                    nc.vector.tensor_add(shifted[:], attn_in[g][:],
                                         negmax[:].broadcast_to([128, 4, 64]))
                    expt = smx.tile([128, 4, 64], f32, tag=f"expt{g}",
                                    name=f"expt{g}")
                    nc.scalar.activation(expt[:], shifted[:],
                                         mybir.ActivationFunctionType.Exp)
                    sums = smx.tile([128, 4], f32, tag=f"sums{g}",
                                    name=f"sums{g}")
                    nc.vector.reduce_sum(sums[:], expt[:],
                                         axis=mybir.AxisListType.X)
                    rsum = smx.tile([128, 4], f32, tag=f"rsum{g}",
                                    name=f"rsum{g}")
                    nc.vector.reciprocal(rsum[:], sums[:])
                    attn_bf = smx.tile([128, 4, 64], bf16, tag=f"attn_bf{g}",
                                       name=f"attn_bf{g}")
                    nc.vector.tensor_mul(attn_bf[:], expt[:],
                                         rsum[:].broadcast_to([128, 4, 64]))

                    # At[g][fc][(parity,j), o] = sum_n attn[n,j] WoutT[f,o]
                    for fc in range(4):
                        ap_t = aps.tile([128, 256], f32, tag="ap_t")
                        for parity in range(2):
                            ps = slice(parity * 64, (parity + 1) * 64)
                            nc.tensor.matmul(ap_t[ps, :], attn_bf[ps, fc, :],
                                             woT[fc][ps, :],
                                             start=True, stop=True)
                        nc.vector.tensor_copy(At[g][fc][:], ap_t[:])

                    # Mt[c, o] = sum_f Wv[f, c] At[f, o]; both c-chunks share
                    # one psum bank (cols 0:256 / 256:512)
                    mp = mps.tile([128, 512], f32, tag="mp")
                    for cchunk in range(2):
                        cs = slice(cchunk * 128, (cchunk + 1) * 128)
                        for fc in range(4):
                            nc.tensor.matmul(
                                mp[:, cchunk * 256:(cchunk + 1) * 256],
                                wv_bf[fc][:, cs], At[g][fc][:],
                                start=(cchunk == 0 and fc == 0),
                                stop=(cchunk == 1 and fc == 3))
                    for cchunk in range(2):
                        eng_copy = (nc.vector.tensor_copy if cchunk == 0
                                    else nc.scalar.copy)
                        eng_copy(Mt[g][cchunk][:],
                                 mp[:, cchunk * 256:(cchunk + 1) * 256])

                    # out[o, m] = sum_c Mt[c, o] X[c, m] + bout
                    for ot in range(2):
                        os_ = slice(ot * 128, (ot + 1) * 128)
                        for mt4 in range(NP):
                            op_t = ops.tile([128, 512], f32, tag="op_t")
                            for cc in range(2):
                                nc.tensor.matmul(op_t[:], Mt[g][cc][:, os_],
                                                 x_bf[g][cc][mt4][:],
                                                 start=(cc == 0), stop=(cc == 1))
                            o_t = osb.tile([128, 512], f32, tag="o_t")
                            if mt4 % 2 == 0:
                                nc.vector.tensor_scalar_add(
                                    o_t[:], op_t[:], bout_sb[:, ot:ot + 1])
                            else:
                                nc.scalar.activation(
                                    o_t[:], op_t[:],
                                    mybir.ActivationFunctionType.Identity,
                                    bias=bout_sb[:, ot:ot + 1])
                            b0 = g * 8 + ot * 4
                            ms = slice(mt4 * 512, (mt4 + 1) * 512)
                            nc.sync.dma_start(
                                out_ext[b0:b0 + 4, :, ms].rearrange(
                                    "a b m -> (a b) m"),
                                o_t[:])

    nc.compile()
    return nc


def _get_nc():
    if "nc" not in _CACHE:
        _CACHE["nc"] = _build_nc()
    return _CACHE["nc"]


def make_in_maps(x, Wq, Wkv, Wout, bout):
    xf = np.ascontiguousarray(x, dtype=np.float32).reshape(B, C, N)
    Wq = np.ascontiguousarray(Wq, dtype=np.float32)
    Wkv = np.ascontiguousarray(Wkv, dtype=np.float32)
    Wout = np.ascontiguousarray(Wout, dtype=np.float32)
    bout = np.ascontiguousarray(bout, dtype=np.float32)
    return [
        {
            "x": np.ascontiguousarray(xf[:, :, i * NS:(i + 1) * NS]),
            "Wq": Wq, "Wkv": Wkv, "Wout": Wout, "bout": bout,
        }
        for i in range(N_CORES)
    ]


def gather_out(results):
    out = np.concatenate([results[i]["out"] for i in range(N_CORES)], axis=2)
    return out.reshape(B, C, HS, WS).astype(np.float32)


def run_sharded(in_maps, **kw):
    from concourse.bass_utils import run_bass_kernel_spmd
    nc = _get_nc()
    return run_bass_kernel_spmd(nc, in_maps, list(range(N_CORES)), **kw)


def kernel(x, Wq, Wkv, Wout, bout):
    in_maps = make_in_maps(x, Wq, Wkv, Wout, bout)
    res = run_sharded(in_maps)
    return gather_out(res.results)


if __name__ == "__main__":
    nc = _get_nc()
    print("built + compiled OK")
